# revision 25
# baseline (speedup 1.0000x reference)
"""Trainium2 Bass kernel for the AssociativeMemoryL1 problem.

out       = hidden + sigmoid(hidden @ Wg.T) * ((q@memory per head) @ Wo.T)
new_mem   = 0.99*memory + sum_tokens k^T v   (per head)

Strategy (8 NeuronCores, data-parallel over the 16384 tokens):
- The readout path is linear in hidden: q@M@Wo.T == hidden @ W_ro.T with
  W_ro = Wo @ concat_h(M_h^T Wq_h) folded on the host. W_ro is scaled by
  512 before fp8 quantization (its elements would otherwise sit in fp8's
  subnormal range); the 1/512 is folded into the output merge.
- One fused fp8 DoubleRow GEMM computes [gate | 512*proj] = x @ [Wg | 512*W_ro]^T
  (K=256 per matmul at the same ~263ns issue spacing as bf16 K=128).
- k,v projections stay bf16 (they feed the graded memory output), with
  per-head outer products packed into single PSUM banks and accumulated
  across token tiles in SBUF.
- new_memory: on-device AllReduce of the k^T v partials on the gpsimd
  queue (isolated so its ~70us latency never blocks compute engines),
  combined with host-pre-decayed 0.99*memory via DMA-accumulate.
"""

import numpy as np
import ml_dtypes

import concourse.bacc as bacc
import concourse.mybir as mybir
import concourse.tile as tile
from concourse.bass_utils import run_bass_kernel_spmd

BF16 = mybir.dt.bfloat16
F8 = mybir.dt.float8e4
F32 = mybir.dt.float32
NPBF = ml_dtypes.bfloat16
NP8 = mybir.dt.np(F8)
DR = mybir.MatmulPerfMode.DoubleRow

N_CORES = 8
B, S, D = 4, 4096, 2048
H, DK, DV = 16, 64, 64
HD = H * DK  # 1024
T = (B * S) // N_CORES  # tokens per core = 2048
NT = T // 128  # 16 token tiles
ND = D // 128  # 16 bf16 contraction chunks
NC2 = D // 256  # 8 fp8 DoubleRow contraction chunks
NTB = T // 512  # 4 512-token blocks
DECAY = 0.99
RO_SCALE = 512.0  # fp8 subnormal-avoidance scale on W_ro

TRACE = False
LAST_RESULT = None

_CACHE = {}


def _r2(ap, width):
    """View a [128, 2*width] tile as the DoubleRow 3D AP [128, 2, width]."""
    return ap.rearrange("p (two w) -> p two w", two=2)


def _build():
    if "nc" in _CACHE:
        return _CACHE["nc"]

    nc = bacc.Bacc("TRN2", target_bir_lowering=False, debug=False,
                   num_devices=N_CORES)

    xt_e = nc.dram_tensor("xt", [ND, 128, T], BF16, kind="ExternalInput")
    xt8_e = nc.dram_tensor("xt8", [NC2, 128, 2 * T], F8, kind="ExternalInput")
    x_e = nc.dram_tensor("x", [T, D], F32, kind="ExternalInput")
    wkv_e = nc.dram_tensor("wkv", [ND, 128, 2048], BF16, kind="ExternalInput")
    wbig8_e = nc.dram_tensor("wbig8", [NC2, 128, 2 * 2 * D], F8,
                             kind="ExternalInput")
    mempd_e = nc.dram_tensor("mempd", [128, 512], F32, kind="ExternalInput")

    out_e = nc.dram_tensor("out", [T, D], F32, kind="ExternalOutput")
    nm_e = nc.dram_tensor("newmem", [128, 512], F32, kind="ExternalOutput")

    ar_in = nc.dram_tensor("ar_in", [128, 512], F32)
    ar_out = nc.dram_tensor("ar_out", [128, 512], F32, addr_space="Shared")

    with tile.TileContext(nc) as tc:
        with (
            tc.tile_pool(name="xt8", bufs=NC2) as pxt8,
            tc.tile_pool(name="wbig8", bufs=NC2) as pwb,
            tc.tile_pool(name="const", bufs=1) as pconst,
        ):
            xt8, wbig8 = [], []

            # ================= P1: k,v projections + memory update ======
            with (
                tc.tile_pool(name="wkv", bufs=ND) as pw1,
                tc.tile_pool(name="xtb", bufs=18) as pxtb,
                tc.tile_pool(name="kvsb", bufs=3) as pkv,
                tc.tile_pool(name="kvps", bufs=7, space="PSUM") as pp1,
                tc.tile_pool(name="memps", bufs=1, space="PSUM") as ppm,
            ):
                macc = [
                    pconst.tile([128, 512], F32, tag="macc0", name="macc0"),
                    pconst.tile([128, 512], F32, tag="macc1", name="macc1"),
                ]
                wkv, xtb = [], {}
                # interleave W_kv and x^T block-0 loads: the first
                # accumulation sweep consumes them pairwise in dc order
                # k-halves of W_kv (+x^T block 0) load first so the
                # ramp unlocks k-chains for ~3 tiles in flight (2 PSUM
                # banks/tile) instead of 1.75 (4 banks/tile); v-halves
                # stream behind them.
                for dc in range(ND):
                    t = pxtb.tile([128, 512], BF16, tag="xtb",
                                  name=f"xtb0_{dc}")
                    nc.scalar.dma_start(t[:], xt_e[dc, :, 0:512])
                    xtb[dc] = t
                    w = pw1.tile([128, 2048], BF16, tag="wkv", name=f"wkv{dc}")
                    nc.scalar.dma_start(w[:, 0:1024], wkv_e[dc, :, 0:1024])
                    wkv.append(w)
                for dc in range(ND):
                    nc.scalar.dma_start(wkv[dc][:, 1024:2048],
                                        wkv_e[dc, :, 1024:2048])
                for tt in range(NT):
                    tb, ti = tt // 4, tt % 4
                    if ti == 0 and tb > 0:  # stream this 512-token block
                        for dc in range(ND):
                            t = pxtb.tile([128, 512], BF16, tag="xtb",
                                          name=f"xtb{tb}_{dc}")
                            nc.scalar.dma_start(
                                t[:], xt_e[dc, :, tb * 512:(tb + 1) * 512])
                            xtb[dc] = t
                    if tt == 4:
                        # PA operands ride the (otherwise idle) sync queue.
                        # Gate the first load on P1's ramp (tt=3 eviction)
                        # so they don't steal HBM bandwidth from the
                        # DMA-paced first sweep; FIFO orders the rest.
                        for kc in range(NC2):
                            t = pxt8.tile([128, 2 * T], F8, tag="xt8",
                                          name=f"xt8_{kc}")
                            dma = nc.sync.dma_start(t[:], xt8_e[kc])
                            if kc == 0:
                                tile.add_dep_helper(
                                    dma.ins, ramp_done.ins, sync=True,
                                    reason="PA loads after P1 DMA ramp")
                            xt8.append(t)
                            t = pwb.tile([128, 4 * D], F8, tag="wbig8",
                                         name=f"wbig8_{kc}")
                            nc.sync.dma_start(t[:], wbig8_e[kc])
                            wbig8.append(t)
                    k_ps = [pp1.tile([128, 512], F32, tag="kvps",
                                     name=f"kps{tt}_{j}") for j in range(2)]
                    for dc in range(ND):
                        lhs = xtb[dc][:, ti * 128:(ti + 1) * 128]
                        st, sp = dc == 0, dc == ND - 1
                        for j in range(2):
                            nc.tensor.matmul(k_ps[j][:], lhs,
                                             wkv[dc][:, j * 512:(j + 1) * 512],
                                             start=st, stop=sp)
                    ksb = pkv.tile([128, 1024], BF16, tag="ksb")
                    vsb = pkv.tile([128, 1024], BF16, tag="vsb")
                    kcopy = nc.vector.tensor_copy(ksb[:, 0:512], k_ps[0][:])
                    if tt == 3:
                        ramp_done = kcopy
                    nc.vector.tensor_copy(ksb[:, 512:1024], k_ps[1][:])
                    v_ps = [pp1.tile([128, 512], F32, tag="kvps",
                                     name=f"vps{tt}_{j}") for j in range(2)]
                    for dc in range(ND):
                        lhs = xtb[dc][:, ti * 128:(ti + 1) * 128]
                        st, sp = dc == 0, dc == ND - 1
                        for j in range(2):
                            nc.tensor.matmul(v_ps[j][:], lhs,
                                             wkv[dc][:, (2 + j) * 512:(3 + j) * 512],
                                             start=st, stop=sp)
                    nc.vector.tensor_copy(vsb[:, 0:512], v_ps[0][:])
                    nc.vector.tensor_copy(vsb[:, 512:1024], v_ps[1][:])
                    # Single-shot packed head outer products (start=True
                    # clears has-written bits for the written partitions
                    # across the whole bank, so cross-tile accumulation of
                    # column-packed regions is unsafe; accumulate in SBUF).
                    mps = ppm.tile([128, 512], F32, tag="memps",
                                   name=f"memps{tt}")
                    for h in range(H):
                        po = (h % 2) * 64
                        fo = (h // 2) * 64
                        nc.tensor.matmul(
                            mps[po:po + 64, fo:fo + 64],
                            ksb[:, h * 64:(h + 1) * 64],
                            vsb[:, h * 64:(h + 1) * 64],
                            start=True, stop=True,
                            skip_group_check=True,
                        )
                    if tt == 0:
                        nc.vector.tensor_copy(macc[0][:], mps[:])
                    else:
                        nc.vector.tensor_add(macc[tt % 2][:],
                                             macc[(tt + 1) % 2][:], mps[:])
                msum = macc[(NT - 1) % 2]

            # new_memory: DMAs on the gpsimd queue so the collective
            # latency never head-of-line blocks compute-engine queues;
            # the final add is emitted at the end of the graph (below).
            nc.gpsimd.dma_start(ar_in[:], msum[:])
            nc.gpsimd.collective_compute(
                "AllReduce", mybir.AluOpType.add,
                ins=[ar_in[:]], outs=[ar_out[:]],
                replica_groups=[list(range(N_CORES))],
            )
            arsb = pconst.tile([128, 512], F32, tag="arsb")
            nc.gpsimd.dma_start(arsb[:], ar_out[:])
            mpsb = pconst.tile([128, 512], F32, tag="mpsb")
            nc.gpsimd.dma_start(mpsb[:], mempd_e[:])

            # ========= PA: fused [gate | 512*proj] fp8 DR GEMM + merge ==
            with (
                tc.tile_pool(name="x3", bufs=4) as px3,
                tc.tile_pool(name="g3", bufs=2) as pg3,
                tc.tile_pool(name="m3", bufs=8) as pm3,
                tc.tile_pool(name="o3", bufs=8) as po3,
                tc.tile_pool(name="gps", bufs=2, space="PSUM") as pgps,
                tc.tile_pool(name="pps", bufs=2, space="PSUM") as ppps,
            ):
                for half in range(2):
                    cofs = half * 1024
                    for tt in range(NT):
                        gps = pgps.tile([128, 1024], F32, tag="gps")
                        pps = ppps.tile([128, 1024], F32, tag="pps")
                        for kc in range(NC2):
                            lhs = _r2(xt8[kc][:], T)[
                                :, :, tt * 128:(tt + 1) * 128]
                            st, sp = kc == 0, kc == NC2 - 1
                            w3 = _r2(wbig8[kc][:], 2 * D)
                            nc.tensor.matmul(
                                gps[:, 0:512], lhs,
                                w3[:, :, cofs:cofs + 512],
                                perf_mode=DR, start=st, stop=sp,
                                skip_group_check=True)
                            nc.tensor.matmul(
                                gps[:, 512:1024], lhs,
                                w3[:, :, cofs + 512:cofs + 1024],
                                perf_mode=DR, start=st, stop=sp,
                                skip_group_check=True)
                            nc.tensor.matmul(
                                pps[:, 0:512], lhs,
                                w3[:, :, D + cofs:D + cofs + 512],
                                perf_mode=DR, start=st, stop=sp,
                                skip_group_check=True)
                            nc.tensor.matmul(
                                pps[:, 512:1024], lhs,
                                w3[:, :, D + cofs + 512:D + cofs + 1024],
                                perf_mode=DR, start=st, stop=sp,
                                skip_group_check=True)
                        gsb = pg3.tile([128, 1024], F32, tag="gsb")
                        nc.scalar.activation(
                            gsb[:], gps[:],
                            mybir.ActivationFunctionType.Sigmoid)
                        xsb = px3.tile([128, 1024], F32, tag="xsb")
                        nc.scalar.dma_start(
                            xsb[:], x_e[tt * 128:(tt + 1) * 128,
                                        cofs:cofs + 1024])
                        # merge in 512-col chunks: out = x + g*(p/512)
                        for ch in range(2):
                            cs = ch * 512
                            msb = pm3.tile([128, 512], F32, tag="msb",
                                           name=f"msb{half}_{tt}_{ch}")
                            nc.vector.scalar_tensor_tensor(
                                msb[:], pps[:, cs:cs + 512], 1.0 / RO_SCALE,
                                gsb[:, cs:cs + 512],
                                op0=mybir.AluOpType.mult,
                                op1=mybir.AluOpType.mult)
                            osb = po3.tile([128, 512], F32, tag="osb",
                                           name=f"osb{half}_{tt}_{ch}")
                            last_merge = nc.vector.tensor_add(
                                osb[:], msb[:], xsb[:, cs:cs + 512])
                            nc.sync.dma_start(
                                out_e[tt * 128:(tt + 1) * 128,
                                      cofs + cs:cofs + cs + 512], osb[:])

            # newmem combine: force it AFTER the last merge in the DVE
            # stream (the scheduler's cost model underestimates the
            # collective latency and would otherwise place this add
            # mid-stream, head-of-line blocking every PA merge on DVE)
            nmsb = pconst.tile([128, 512], F32, tag="nmsb")
            nm_add = nc.vector.tensor_add(nmsb[:], mpsb[:], arsb[:])
            tile.add_dep_helper(
                nm_add.ins,
                last_merge.ins,
                sync=False, reason="newmem add after all PA merges")
            nc.gpsimd.dma_start(nm_e[:], nmsb[:])

    nc.compile()
    _CACHE["nc"] = nc
    return nc


def _dr_stage(wT, nchunks, width):
    """[K, width] -> DoubleRow-paired fp8 [nchunks, 128, 2*width]."""
    return np.ascontiguousarray(
        wT.reshape(nchunks, 2, 128, width).transpose(0, 2, 1, 3)
    ).astype(NP8).reshape(nchunks, 128, 2 * width)


def _stage(hidden, memory, Wk, Wv, Wq, Wg, Wo):
    hs = np.ascontiguousarray(hidden.reshape(B * S, D))
    wkv = np.concatenate([Wk.T, Wv.T], axis=1).astype(NPBF).reshape(ND, 128, 2048)
    # fold q @ memory @ Wo.T into a single linear map (the readout uses
    # the PRE-update memory, so this is exact)
    Wro1 = np.concatenate(
        [memory[h].T @ Wq[h * 64:(h + 1) * 64] for h in range(H)], 0)
    W_ro = Wo @ Wro1  # [D, D]
    wbigT = np.concatenate([Wg.T, W_ro.T * RO_SCALE], axis=1)  # [D, 2D]
    wbig8 = _dr_stage(wbigT, NC2, 2 * D)
    mempd = np.zeros((128, 512), np.float32)
    for h in range(H):
        mempd[(h % 2) * 64:(h % 2) * 64 + 64,
              (h // 2) * 64:(h // 2) * 64 + 64] = DECAY * memory[h]

    in_maps = []
    for i in range(N_CORES):
        shard = hs[i * T:(i + 1) * T]
        xT = shard.T
        in_maps.append({
            "xt": xT.astype(NPBF).reshape(ND, 128, T),
            "xt8": _dr_stage(xT, NC2, T),
            "x": np.ascontiguousarray(shard, dtype=np.float32),
            "wkv": wkv, "wbig8": wbig8, "mempd": mempd,
        })
    return in_maps


def kernel(hidden, memory, Wk, Wv, Wq, Wg, Wo):
    global LAST_RESULT
    nc = _build()
    in_maps = _stage(hidden, memory, Wk, Wv, Wq, Wg, Wo)

    kwargs = {}
    if TRACE:
        try:  # install NTFF profile hook if absent (best effort)
            import importlib.util
            import sys
            import types
            if "antenv.axon_hooks" not in sys.modules:
                spec = importlib.util.spec_from_file_location(
                    "_trn_boot", "/root/.axon_site/trn_agent_boot/trn_boot.py")
                boot = importlib.util.module_from_spec(spec)
                spec.loader.exec_module(boot)
                hook = boot._ntff_profile_via_ctypes("/opt/axon/libaxon_pjrt.so")
                mod = types.ModuleType("antenv.axon_hooks")
                mod._HOOK = hook
                mod.set_axon_ntff_profile_hook = lambda h: setattr(mod, "_HOOK", h)
                mod.get_axon_ntff_profile_hook = lambda: mod._HOOK
                sys.modules["antenv.axon_hooks"] = mod
                import antenv
                antenv.axon_hooks = mod
            kwargs["trace"] = True
        except Exception:
            pass

    res = run_bass_kernel_spmd(nc, in_maps, core_ids=list(range(N_CORES)),
                               **kwargs)
    LAST_RESULT = res

    out = np.concatenate([res.results[i]["out"] for i in range(N_CORES)],
                         axis=0).reshape(B, S, D)
    nm_p = res.results[0]["newmem"]
    new_memory = np.empty((H, DK, DV), np.float32)
    for h in range(H):
        new_memory[h] = nm_p[(h % 2) * 64:(h % 2) * 64 + 64,
                             (h // 2) * 64:(h // 2) * 64 + 64]
    return out, new_memory


# revision 26
# speedup vs baseline: 1.0415x; 1.0415x over previous
"""Trainium2 Bass kernel for the AssociativeMemoryL1 problem.

out       = hidden + sigmoid(hidden @ Wg.T) * ((q@memory per head) @ Wo.T)
new_mem   = 0.99*memory + sum_tokens k^T v   (per head)

Strategy (8 NeuronCores, data-parallel over the 16384 tokens):
- The readout path is linear in hidden: q@M@Wo.T == hidden @ W_ro.T with
  W_ro = Wo @ concat_h(M_h^T Wq_h) folded on the host. W_ro is scaled by
  512 before fp8 quantization (its elements would otherwise sit in fp8's
  subnormal range); the 1/512 is folded into the output merge.
- One fused fp8 DoubleRow GEMM computes [gate | 512*proj] = x @ [Wg | 512*W_ro]^T
  (K=256 per matmul at the same ~263ns issue spacing as bf16 K=128).
- k,v projections stay bf16 (they feed the graded memory output), with
  per-head outer products packed into single PSUM banks and accumulated
  across token tiles in SBUF.
- new_memory: on-device AllReduce of the k^T v partials on the gpsimd
  queue (isolated so its ~70us latency never blocks compute engines),
  combined with host-pre-decayed 0.99*memory via DMA-accumulate.
"""

import numpy as np
import ml_dtypes

import concourse.bacc as bacc
import concourse.mybir as mybir
import concourse.tile as tile
from concourse.bass_utils import run_bass_kernel_spmd

BF16 = mybir.dt.bfloat16
F8 = mybir.dt.float8e4
F32 = mybir.dt.float32
NPBF = ml_dtypes.bfloat16
NP8 = mybir.dt.np(F8)
DR = mybir.MatmulPerfMode.DoubleRow

N_CORES = 8
B, S, D = 4, 4096, 2048
H, DK, DV = 16, 64, 64
HD = H * DK  # 1024
T = (B * S) // N_CORES  # tokens per core = 2048
NT = T // 128  # 16 token tiles
ND = D // 128  # 16 bf16 contraction chunks
NC2 = D // 256  # 8 fp8 DoubleRow contraction chunks
NTB = T // 512  # 4 512-token blocks
DECAY = 0.99
RO_SCALE = 512.0  # fp8 subnormal-avoidance scale on W_ro

TRACE = False
LAST_RESULT = None

_CACHE = {}


def _r2(ap, width):
    """View a [128, 2*width] tile as the DoubleRow 3D AP [128, 2, width]."""
    return ap.rearrange("p (two w) -> p two w", two=2)


def _build():
    if "nc" in _CACHE:
        return _CACHE["nc"]

    nc = bacc.Bacc("TRN2", target_bir_lowering=False, debug=False,
                   num_devices=N_CORES)

    xt_e = nc.dram_tensor("xt", [ND, 128, T], BF16, kind="ExternalInput")
    xt8_e = nc.dram_tensor("xt8", [NC2, 128, 2 * T], F8, kind="ExternalInput")
    x_e = nc.dram_tensor("x", [T, D], F32, kind="ExternalInput")
    wkv_e = nc.dram_tensor("wkv", [ND, 128, 2048], BF16, kind="ExternalInput")
    wbig8_e = nc.dram_tensor("wbig8", [NC2, 128, 2 * 2 * D], F8,
                             kind="ExternalInput")
    mempd_e = nc.dram_tensor("mempd", [128, 512], F32, kind="ExternalInput")

    out_e = nc.dram_tensor("out", [T, D], F32, kind="ExternalOutput")
    nm_e = nc.dram_tensor("newmem", [128, 512], F32, kind="ExternalOutput")

    ar_in = nc.dram_tensor("ar_in", [128, 512], F32)
    ar_out = nc.dram_tensor("ar_out", [128, 512], F32, addr_space="Shared")

    with tile.TileContext(nc) as tc:
        with (
            tc.tile_pool(name="xt8", bufs=NC2) as pxt8,
            tc.tile_pool(name="wbig8", bufs=NC2) as pwb,
            tc.tile_pool(name="const", bufs=1) as pconst,
        ):
            xt8, wbig8 = [], []

            # ================= P1: k,v projections + memory update ======
            with (
                tc.tile_pool(name="wkv", bufs=ND) as pw1,
                tc.tile_pool(name="xtb", bufs=18) as pxtb,
                tc.tile_pool(name="kvsb", bufs=3) as pkv,
                tc.tile_pool(name="kvps", bufs=7, space="PSUM") as pp1,
                tc.tile_pool(name="memps", bufs=1, space="PSUM") as ppm,
            ):
                macc = [
                    pconst.tile([128, 512], F32, tag="macc0", name="macc0"),
                    pconst.tile([128, 512], F32, tag="macc1", name="macc1"),
                ]
                wkv, xtb = [], {}
                # interleave W_kv and x^T block-0 loads: the first
                # accumulation sweep consumes them pairwise in dc order
                for dc in range(ND):
                    w = pw1.tile([128, 2048], BF16, tag="wkv", name=f"wkv{dc}")
                    nc.scalar.dma_start(w[:], wkv_e[dc])
                    wkv.append(w)
                    t = pxtb.tile([128, 512], BF16, tag="xtb",
                                  name=f"xtb0_{dc}")
                    nc.scalar.dma_start(t[:], xt_e[dc, :, 0:512])
                    xtb[dc] = t
                for tt in range(NT):
                    tb, ti = tt // 4, tt % 4
                    if ti == 0 and tb > 0:  # stream this 512-token block
                        for dc in range(ND):
                            t = pxtb.tile([128, 512], BF16, tag="xtb",
                                          name=f"xtb{tb}_{dc}")
                            nc.scalar.dma_start(
                                t[:], xt_e[dc, :, tb * 512:(tb + 1) * 512])
                            xtb[dc] = t
                    if tt == 4:
                        # PA operands ride the (otherwise idle) sync queue.
                        # Gate the first load on P1's ramp (tt=3 eviction)
                        # so they don't steal HBM bandwidth from the
                        # DMA-paced first sweep; FIFO orders the rest.
                        for kc in range(NC2):
                            t = pxt8.tile([128, 2 * T], F8, tag="xt8",
                                          name=f"xt8_{kc}")
                            dma = nc.sync.dma_start(t[:], xt8_e[kc])
                            if kc == 0:
                                tile.add_dep_helper(
                                    dma.ins, ramp_done.ins, sync=True,
                                    reason="PA loads after P1 DMA ramp")
                            xt8.append(t)
                            t = pwb.tile([128, 4 * D], F8, tag="wbig8",
                                         name=f"wbig8_{kc}")
                            nc.sync.dma_start(t[:], wbig8_e[kc])
                            wbig8.append(t)
                    kv_ps = [pp1.tile([128, 512], F32, tag="kvps",
                                      name=f"kvps{tt}_{j}") for j in range(4)]
                    for dc in range(ND):
                        lhs = xtb[dc][:, ti * 128:(ti + 1) * 128]
                        st, sp = dc == 0, dc == ND - 1
                        for j in range(4):
                            nc.tensor.matmul(kv_ps[j][:], lhs,
                                             wkv[dc][:, j * 512:(j + 1) * 512],
                                             start=st, stop=sp)
                    ksb = pkv.tile([128, 1024], BF16, tag="ksb")
                    vsb = pkv.tile([128, 1024], BF16, tag="vsb")
                    kcopy = nc.vector.tensor_copy(ksb[:, 0:512], kv_ps[0][:])
                    if tt == 3:
                        ramp_done = kcopy
                    nc.vector.tensor_copy(ksb[:, 512:1024], kv_ps[1][:])
                    nc.vector.tensor_copy(vsb[:, 0:512], kv_ps[2][:])
                    nc.vector.tensor_copy(vsb[:, 512:1024], kv_ps[3][:])
                    # Single-shot packed head outer products (start=True
                    # clears has-written bits for the written partitions
                    # across the whole bank, so cross-tile accumulation of
                    # column-packed regions is unsafe; accumulate in SBUF).
                    mps = ppm.tile([128, 512], F32, tag="memps",
                                   name=f"memps{tt}")
                    for h in range(H):
                        po = (h % 2) * 64
                        fo = (h // 2) * 64
                        nc.tensor.matmul(
                            mps[po:po + 64, fo:fo + 64],
                            ksb[:, h * 64:(h + 1) * 64],
                            vsb[:, h * 64:(h + 1) * 64],
                            start=True, stop=True,
                            skip_group_check=True,
                        )
                    if tt == 0:
                        nc.vector.tensor_copy(macc[0][:], mps[:])
                    else:
                        nc.vector.tensor_add(macc[tt % 2][:],
                                             macc[(tt + 1) % 2][:], mps[:])
                msum = macc[(NT - 1) % 2]

            # new_memory: DMAs on the gpsimd queue so the collective
            # latency never head-of-line blocks compute-engine queues;
            # the final add is emitted at the end of the graph (below).
            nc.gpsimd.dma_start(ar_in[:], msum[:])
            nc.gpsimd.collective_compute(
                "AllReduce", mybir.AluOpType.add,
                ins=[ar_in[:]], outs=[ar_out[:]],
                replica_groups=[list(range(N_CORES))],
            )
            arsb = pconst.tile([128, 512], F32, tag="arsb")
            nc.gpsimd.dma_start(arsb[:], ar_out[:])
            mpsb = pconst.tile([128, 512], F32, tag="mpsb")
            nc.gpsimd.dma_start(mpsb[:], mempd_e[:])

            # ========= PA: fused [gate | 512*proj] fp8 DR GEMM + merge ==
            with (
                tc.tile_pool(name="x3", bufs=4) as px3,
                tc.tile_pool(name="g3", bufs=2) as pg3,
                tc.tile_pool(name="m3", bufs=8) as pm3,
                tc.tile_pool(name="o3", bufs=8) as po3,
                tc.tile_pool(name="gps", bufs=2, space="PSUM") as pgps,
                tc.tile_pool(name="pps", bufs=2, space="PSUM") as ppps,
            ):
                for half in range(2):
                    cofs = half * 1024
                    for tt in range(NT):
                        gps = pgps.tile([128, 1024], F32, tag="gps")
                        pps = ppps.tile([128, 1024], F32, tag="pps")
                        for kc in range(NC2):
                            lhs = _r2(xt8[kc][:], T)[
                                :, :, tt * 128:(tt + 1) * 128]
                            st, sp = kc == 0, kc == NC2 - 1
                            w3 = _r2(wbig8[kc][:], 2 * D)
                            nc.tensor.matmul(
                                gps[:, 0:512], lhs,
                                w3[:, :, cofs:cofs + 512],
                                perf_mode=DR, start=st, stop=sp,
                                skip_group_check=True)
                            nc.tensor.matmul(
                                gps[:, 512:1024], lhs,
                                w3[:, :, cofs + 512:cofs + 1024],
                                perf_mode=DR, start=st, stop=sp,
                                skip_group_check=True)
                            nc.tensor.matmul(
                                pps[:, 0:512], lhs,
                                w3[:, :, D + cofs:D + cofs + 512],
                                perf_mode=DR, start=st, stop=sp,
                                skip_group_check=True)
                            nc.tensor.matmul(
                                pps[:, 512:1024], lhs,
                                w3[:, :, D + cofs + 512:D + cofs + 1024],
                                perf_mode=DR, start=st, stop=sp,
                                skip_group_check=True)
                        gsb = pg3.tile([128, 1024], F32, tag="gsb")
                        nc.scalar.activation(
                            gsb[:], gps[:],
                            mybir.ActivationFunctionType.Sigmoid)
                        xsb = px3.tile([128, 1024], F32, tag="xsb")
                        nc.scalar.dma_start(
                            xsb[:], x_e[tt * 128:(tt + 1) * 128,
                                        cofs:cofs + 1024])
                        # merge in 512-col chunks: out = x + g*(p/512)
                        for ch in range(2):
                            cs = ch * 512
                            msb = pm3.tile([128, 512], F32, tag="msb",
                                           name=f"msb{half}_{tt}_{ch}")
                            nc.vector.scalar_tensor_tensor(
                                msb[:], pps[:, cs:cs + 512], 1.0 / RO_SCALE,
                                gsb[:, cs:cs + 512],
                                op0=mybir.AluOpType.mult,
                                op1=mybir.AluOpType.mult)
                            osb = po3.tile([128, 512], F32, tag="osb",
                                           name=f"osb{half}_{tt}_{ch}")
                            last_merge = nc.vector.tensor_add(
                                osb[:], msb[:], xsb[:, cs:cs + 512])
                            nc.sync.dma_start(
                                out_e[tt * 128:(tt + 1) * 128,
                                      cofs + cs:cofs + cs + 512], osb[:])

            # newmem combine: force it AFTER the last merge in the DVE
            # stream (the scheduler's cost model underestimates the
            # collective latency and would otherwise place this add
            # mid-stream, head-of-line blocking every PA merge on DVE)
            nmsb = pconst.tile([128, 512], F32, tag="nmsb")
            nm_add = nc.vector.tensor_add(nmsb[:], mpsb[:], arsb[:])
            tile.add_dep_helper(
                nm_add.ins,
                last_merge.ins,
                sync=False, reason="newmem add after all PA merges")
            nc.gpsimd.dma_start(nm_e[:], nmsb[:])

    nc.compile()
    _CACHE["nc"] = nc
    return nc


def _dr_stage(wT, nchunks, width):
    """[K, width] -> DoubleRow-paired fp8 [nchunks, 128, 2*width]."""
    return np.ascontiguousarray(
        wT.reshape(nchunks, 2, 128, width).transpose(0, 2, 1, 3)
    ).astype(NP8).reshape(nchunks, 128, 2 * width)


def _stage(hidden, memory, Wk, Wv, Wq, Wg, Wo):
    hs = np.ascontiguousarray(hidden.reshape(B * S, D))
    wkv = np.concatenate([Wk.T, Wv.T], axis=1).astype(NPBF).reshape(ND, 128, 2048)
    # fold q @ memory @ Wo.T into a single linear map (the readout uses
    # the PRE-update memory, so this is exact)
    Wro1 = np.concatenate(
        [memory[h].T @ Wq[h * 64:(h + 1) * 64] for h in range(H)], 0)
    W_ro = Wo @ Wro1  # [D, D]
    wbigT = np.concatenate([Wg.T, W_ro.T * RO_SCALE], axis=1)  # [D, 2D]
    wbig8 = _dr_stage(wbigT, NC2, 2 * D)
    mempd = np.zeros((128, 512), np.float32)
    for h in range(H):
        mempd[(h % 2) * 64:(h % 2) * 64 + 64,
              (h // 2) * 64:(h // 2) * 64 + 64] = DECAY * memory[h]

    in_maps = []
    for i in range(N_CORES):
        shard = hs[i * T:(i + 1) * T]
        xT = shard.T
        in_maps.append({
            "xt": xT.astype(NPBF).reshape(ND, 128, T),
            "xt8": _dr_stage(xT, NC2, T),
            "x": np.ascontiguousarray(shard, dtype=np.float32),
            "wkv": wkv, "wbig8": wbig8, "mempd": mempd,
        })
    return in_maps


def kernel(hidden, memory, Wk, Wv, Wq, Wg, Wo):
    global LAST_RESULT
    nc = _build()
    in_maps = _stage(hidden, memory, Wk, Wv, Wq, Wg, Wo)

    kwargs = {}
    if TRACE:
        try:  # install NTFF profile hook if absent (best effort)
            import importlib.util
            import sys
            import types
            if "antenv.axon_hooks" not in sys.modules:
                spec = importlib.util.spec_from_file_location(
                    "_trn_boot", "/root/.axon_site/trn_agent_boot/trn_boot.py")
                boot = importlib.util.module_from_spec(spec)
                spec.loader.exec_module(boot)
                hook = boot._ntff_profile_via_ctypes("/opt/axon/libaxon_pjrt.so")
                mod = types.ModuleType("antenv.axon_hooks")
                mod._HOOK = hook
                mod.set_axon_ntff_profile_hook = lambda h: setattr(mod, "_HOOK", h)
                mod.get_axon_ntff_profile_hook = lambda: mod._HOOK
                sys.modules["antenv.axon_hooks"] = mod
                import antenv
                antenv.axon_hooks = mod
            kwargs["trace"] = True
        except Exception:
            pass

    res = run_bass_kernel_spmd(nc, in_maps, core_ids=list(range(N_CORES)),
                               **kwargs)
    LAST_RESULT = res

    out = np.concatenate([res.results[i]["out"] for i in range(N_CORES)],
                         axis=0).reshape(B, S, D)
    nm_p = res.results[0]["newmem"]
    new_memory = np.empty((H, DK, DV), np.float32)
    for h in range(H):
        new_memory[h] = nm_p[(h % 2) * 64:(h % 2) * 64 + 64,
                             (h // 2) * 64:(h // 2) * 64 + 64]
    return out, new_memory
